# revision 34
# baseline (speedup 1.0000x reference)
"""DynamicMoE (B=4, S=2048, D=1024, E=8, H=4096, top-2) on 8 trn2 cores.

Key observation: the reference loops experts in index order and OVERWRITES
(out = where(w_i>0, y_i, out)), so each token's final output comes from the
single highest-indexed expert of its top-2. Each token therefore needs exactly
one expert MLP, with its input pre-scaled by that expert's softmax score.

Strategy (expert-parallel with host-side routing):
- Host: route in f64, scale+permute tokens by expert, pack (expert, token
  block) parts into 8 cores x NSLOT weight slots via a small DP packer.
- Device (one SPMD program): per slot, a 2-layer MLP in bf16 (full PE rate,
  ~4e-3 rel err, half the HBM traffic of f32) with tokens on the matmul
  moving dim, weights streamed from HBM in pre-transposed layouts so all
  DMAs are contiguous.
"""

import numpy as np
import ml_dtypes

_B, _S, _D, _E, _H = 4, 2048, 1024, 8, 4096
_N = _B * _S
_KD = _D // 128   # 8 d-chunks (layer-1 contraction / layer-2 output)
_HI = _H // 128   # 32 h-chunks

_BF16 = ml_dtypes.bfloat16

# empirical per-core constants (measured from baseline traces)
_PE_NS_TOK = 213.4          # 512 rows/token @ 2.4 GHz
_MM_OVH_NS = 5.0            # fixed per-matmul overhead (bf16, measured)
_MM_PER_CHUNK = 512         # matmuls per token chunk (256 L1 + 256 L2)
_DMA_BPS = 358e9
_W_BYTES_SLOT = 2 * _H * _D * 2   # w1+w2 bf16 per slot


def _chunks(L, first_small=False):
    """Split L tokens into matmul chunks of <=496 (PSUM bank minus margin).
    first_small carves a small leading chunk so the slot's first matmul
    waits on as little DMA as possible (kernel startup)."""
    pos = []
    t0 = 0
    if first_small and L > 624:
        pos.append((0, 128))
        t0 = 128
    rem = L - t0
    n = max(1, -(-rem // 496))
    base, extra = divmod(rem, n)
    for i in range(n):
        c = base + (1 if i < extra else 0)
        pos.append((t0, c))
        t0 += c
    return pos


def _route(x, gate_w, gate_b):
    """Per-token (expert, scale): the higher-indexed of the top-2 experts and
    its softmax score. f64 to track the f32 reference's ordering closely."""
    xf = x.reshape(_N, _D).astype(np.float64)
    logits = xf @ gate_w.astype(np.float64).T + gate_b.astype(np.float64)
    # jax.lax.top_k tie-break: smaller index first -> stable descending sort
    top2 = np.argsort(-logits, axis=1, kind="stable")[:, :2]
    e_sel = top2.max(axis=1)
    m = logits.max(axis=1, keepdims=True)
    p = np.exp(logits - m)
    p /= p.sum(axis=1, keepdims=True)
    scale = p[np.arange(_N), e_sel]
    return e_sel.astype(np.int64), scale.astype(np.float32)


def _score(sizes):
    """Predicted per-core ns for a slot structure (all slots padded full)."""
    cap = sum(sizes)
    nck = sum(len(_chunks(s)) for s in sizes)
    pe_ns = cap * _PE_NS_TOK + nck * _MM_PER_CHUNK * _MM_OVH_NS
    dma_ns = (len(sizes) * _W_BYTES_SLOT + cap * 6144) / _DMA_BPS * 1e9
    return max(pe_ns, dma_ns)


def _quick_infeasible(counts, sizes):
    """Cheap relaxation: per-expert minimum waste ignoring the 8-slot-per-size
    budget; if even that exceeds the global slack, the candidate is dead."""
    slack = 8 * sum(sizes) - int(sum(counts))
    total = 0
    for e in range(_E):
        n = int(counts[e])
        if n == 0:
            continue
        best = None
        for k0 in range(9):
            for k1 in range(9):
                for k2 in range(9 if len(sizes) > 2 else 1):
                    cov = k0 * sizes[0] + k1 * sizes[1] + (
                        k2 * sizes[2] if len(sizes) > 2 else 0)
                    if cov >= n and (best is None or cov - n < best):
                        best = cov - n
        if best is None:
            return True
        total += best
        if total > slack:
            return True
    return False


def _feasible(counts, sizes):
    """DP: can counts be covered by 8 slots of each size? Returns the
    assignment [(expert, (k_per_size...))] or None."""
    from itertools import product

    experts = [e for e in range(_E) if counts[e] > 0]
    nslot = len(sizes)

    def expert_opts(n):
        opts = [
            t for t in product(range(9), repeat=nslot)
            if sum(k * s for k, s in zip(t, sizes)) >= n
        ]
        return sorted(opts, key=lambda t: (sum(k * s for k, s in zip(t, sizes)), sum(t)))[:96]

    states = {tuple([0] * nslot): []}
    for e in experts:
        nxt = {}
        for opt in expert_opts(int(counts[e])):
            for st, hist in states.items():
                ns = tuple(a + b for a, b in zip(st, opt))
                if all(v <= 8 for v in ns) and ns not in nxt:
                    nxt[ns] = hist + [(e, opt)]
        if not nxt:
            return None
        states = nxt
    return min(states.values(), key=len)


def _pack(counts):
    """Pick the feasible slot structure with the best predicted time."""
    cands = set()
    for A in range(256, 713, 8):
        for Bv in range(256, A + 1, 8):
            if 1000 <= A + Bv <= 1250:
                cands.add((A, Bv))
    # slot sizes in [224, 448]: below 224 the weight-stream density (fixed
    # 16.8MB over a window ~ slot size) spikes past what HBM can deliver;
    # above ~448 the near-full PSUM bank costs ~+8ns per matmul (measured
    # 484-wide at +11ns/mm vs 416-wide at +3ns/mm)
    for A in range(224, 449, 4):
        for Bv in range(224, A + 1, 4):
            for Cv in range(224, Bv + 1, 4):
                if 1024 <= A + Bv + Cv <= 1100:
                    cands.add((A, Bv, Cv))
    # guaranteed-feasible fallbacks
    cands.add((704, 704))
    cands.add((704, 704, 704))

    alloc = None
    for sizes in sorted(cands, key=_score):
        if _quick_infeasible(counts, list(sizes)):
            continue
        alloc = _feasible(counts, list(sizes))
        if alloc is not None:
            break
    assert alloc is not None, f"no feasible slot structure for counts={counts}"

    # largest slot first: small slots are the most DMA-dense (same weight
    # bytes over a shorter compute window), so run them last when their
    # weights can be deep-prefetched during the preceding slot's compute
    perm = sorted(range(len(sizes)), key=lambda i: -sizes[i])
    sizes = tuple(sizes[i] for i in perm)
    alloc = [(e, tuple(opt[i] for i in perm)) for e, opt in alloc]

    # materialize parts: per slot-kind, hand out slot indices core 0..7
    next_core = [0] * len(sizes)
    parts = []
    for e, opt in alloc:
        rem = int(counts[e])
        order = sorted(range(len(sizes)), key=lambda i: -sizes[i])
        for i in order:
            for _ in range(opt[i]):
                take = max(0, min(rem, sizes[i]))
                core = next_core[i]
                next_core[i] += 1
                parts.append((e, take, core, i))
                rem -= take
        assert rem <= 0
    return list(sizes), parts


_PROG_CACHE = {}


def _build_program(slot_sizes):
    """One SPMD Bass program for all 8 cores, parameterized by slot sizes."""
    import concourse.tile as tile
    from concourse import bacc, mybir

    key = tuple(slot_sizes)
    if key in _PROG_CACHE:
        return _PROG_CACHE[key]

    F32 = mybir.dt.float32
    BF16 = mybir.dt.bfloat16
    CAP = sum(slot_sizes)
    nslot = len(slot_sizes)

    nc = bacc.Bacc("TRN2", target_bir_lowering=False, debug=False, num_devices=8)
    xt = nc.dram_tensor("xt", [128, _KD, CAP], BF16, kind="ExternalInput").ap()
    w1d = [
        nc.dram_tensor(f"w1_{s}", [_HI, 128, _KD, 128], BF16, kind="ExternalInput").ap()
        for s in range(nslot)
    ]
    w2d = [
        nc.dram_tensor(f"w2_{s}", [_KD, 128, _HI, 128], BF16, kind="ExternalInput").ap()
        for s in range(nslot)
    ]
    b1d = [
        nc.dram_tensor(f"b1_{s}", [128, _HI], F32, kind="ExternalInput").ap()
        for s in range(nslot)
    ]
    b2d = [
        nc.dram_tensor(f"b2_{s}", [128, _KD], F32, kind="ExternalInput").ap()
        for s in range(nslot)
    ]
    outT = nc.dram_tensor("outT", [_KD, 128, CAP], F32, kind="ExternalOutput").ap()

    Relu = mybir.ActivationFunctionType.Relu
    Ident = mybir.ActivationFunctionType.Identity

    ck_of = [_chunks(Ls, first_small=(s == 0)) for s, Ls in enumerate(slot_sizes)]
    max_cks = max(len(c) for c in ck_of)
    offs = [0]
    for Ls in slot_sizes:
        offs.append(offs[-1] + Ls)
    with tile.TileContext(nc) as tc:
        with tc.tile_pool(name="xp", bufs=max(3, max_cks)) as xp, \
             tc.tile_pool(name="w1p", bufs=16) as w1p, \
             tc.tile_pool(name="w1f", bufs=2) as w1f, \
             tc.tile_pool(name="w2p", bufs=16) as w2p, \
             tc.tile_pool(name="h1p", bufs=2) as h1p, \
             tc.tile_pool(name="cp", bufs=4) as cp, \
             tc.tile_pool(name="wp", bufs=1) as wp, \
             tc.tile_pool(name="op", bufs=6) as op, \
             tc.tile_pool(name="ps1", bufs=4, space="PSUM") as ps1, \
             tc.tile_pool(name="ps2", bufs=4, space="PSUM") as ps2:

            # PE warm-up: the tensor engine p-state ramps to full clock only
            # after ~3us of continuous execution. Burn that ramp on dummy
            # matmuls over a never-written scratch tile (no data dependency,
            # so they start right after sequencer init, ~3.5us) so the PE is
            # at full clock when the first real transfers land (~9.4us).
            # Results land in PSUM banks that real groups later reset via
            # start_tensor_calc, and are never read.
            wt = wp.tile([128, 512], BF16, tag="wrm")
            nc.vector.memset(wt[:], 0)
            for _ in range(8):
                psw = ps2.tile([128, 512], F32, tag="ps2")
                nc.tensor.matmul(psw[:], wt[:, 0:128], wt[:],
                                 start=True, stop=True)

            slot_pre = {}  # s -> (x tiles, w1-hi0 tile, b1 tile, b2 tile)

            def emit_slot_prefetch(s):
                """Queue slot s's x chunks, first w1 block, and biases on the
                sync ring. Order: x ck0, w1 hi0, biases, remaining x chunks —
                so the first matmul of the slot waits on as little as
                possible. For slot 0 (kernel startup, DMA-starved) the later
                x chunks are deferred and interleaved as k-halves between the
                first w1 blocks inside the L1 loop."""
                off = offs[s]
                cks = ck_of[s]
                xc0 = xp.tile([128, _KD, cks[0][1]], BF16, tag="x")
                w1_0 = w1f.tile([128, _KD, 128], BF16, tag="w1first")
                pending = {}  # hi -> [(sbuf slice, dram slice)] for slot 0
                if s == 0:
                    # kernel startup: k-slice the transfers the first PSUM
                    # group needs (w1_0 and the first half of x) into small
                    # DMAs riding parallel queues; x k4..7 (needed only by
                    # matmuls 5-8) is deferred behind the hi=1..4 w1 blocks
                    # via `pending` so those blocks aren't queue-starved
                    for k in range(_KD):
                        nc.sync.dma_start(w1_0[:, k, :], w1d[s][0, :, k, :])
                        if k < 4:
                            nc.sync.dma_start(
                                xc0[:, k, :], xt[:, k, off:off + cks[0][1]]
                            )
                        else:
                            pending.setdefault(k - 3, []).append((
                                xc0[:, k, :],
                                xt[:, k, off:off + cks[0][1]],
                            ))
                else:
                    for kq in range(0, _KD, 2):
                        nc.sync.dma_start(
                            xc0[:, kq:kq + 2, :],
                            xt[:, kq:kq + 2, off:off + cks[0][1]],
                        )
                    nc.sync.dma_start(w1_0[:], w1d[s][0])
                b1_sb = cp.tile([128, _HI], F32, tag="b1")
                nc.sync.dma_start(b1_sb[:], b1d[s][:])
                b2_sb = cp.tile([128, _KD], F32, tag="b2")
                nc.sync.dma_start(b2_sb[:], b2d[s][:])
                xs = [xc0]
                half = _KD // 2
                for ci, (t0, tl) in enumerate(cks[1:], start=1):
                    xc = xp.tile([128, _KD, tl], BF16, tag="x")
                    src = xt[:, :, off + t0:off + t0 + tl]
                    if s == 0:
                        # both halves queued right after the w1-hi1 block:
                        # the PE needs w1-hi1 before any of chunk 1's x
                        pending.setdefault(ci, []).append(
                            (xc[:, 0:half, :], src[:, 0:half, :])
                        )
                        pending.setdefault(ci, []).append(
                            (xc[:, half:_KD, :], src[:, half:_KD, :])
                        )
                    else:
                        nc.sync.dma_start(xc[:], src)
                    xs.append(xc)
                slot_pre[s] = (xs, w1_0, b1_sb, b2_sb, pending)

            slot_w1 = {}

            def stage_w1(s):
                """Issue all remaining w1 block DMAs for slot s in hi order.
                The 24-deep pool throttles them naturally; issuing the whole
                slot up front lets DMA run ahead during the previous slot's
                compute so small (DMA-dense) slots never starve."""
                pending = slot_pre[s][4]
                tiles = {}
                half = _KD // 2
                for hi in range(1, _HI):
                    w1_sb = w1p.tile([128, _KD, 128], BF16, tag="w1")
                    if s == 0 and hi <= 4:
                        # startup: halve the first blocks so each rides two
                        # queues and lands before its PSUM group needs it
                        nc.sync.dma_start(
                            w1_sb[:, 0:half, :], w1d[s][hi, :, 0:half, :])
                        nc.sync.dma_start(
                            w1_sb[:, half:_KD, :], w1d[s][hi, :, half:_KD, :])
                    else:
                        nc.sync.dma_start(w1_sb[:], w1d[s][hi])
                    tiles[hi] = w1_sb
                    # slot 0: deferred x k-slices ride between the first
                    # w1 blocks so the PE is never starved
                    for dst, src in pending.pop(hi, ()):
                        nc.sync.dma_start(dst, src)
                slot_w1[s] = tiles

            slot_w2 = {}

            def stage_w2_di(s, di):
                """Issue one d-chunk of slot s's w2 (4 quarter blocks)."""
                qs = []
                for q in range(4):
                    wq = w2p.tile([128, 8, 128], BF16, tag="w2")
                    nc.sync.dma_start(
                        wq[:], w2d[s][di, :, 8 * q:8 * q + 8, :]
                    )
                    qs.append(wq)
                slot_w2.setdefault(s, {})[di] = qs

            def stage_w2(s):
                """Issue all of slot s's w2 quarter-block DMAs in di order."""
                for di in range(_KD):
                    stage_w2_di(s, di)

            emit_slot_prefetch(0)
            stage_w1(0)
            for s, Ls in enumerate(slot_sizes):
                off = offs[s]
                cks = ck_of[s]
                x_sb, w1_first, b1_sb, b2_sb, pending_x = slot_pre.pop(s)

                h1_sb = h1p.tile([128, _HI, Ls], BF16, tag="h1")
                # (hi=0, chunk>0) runs at the END of layer 1: at slot start
                # only chunk 0's x has landed, so starting with (0,0) alone
                # avoids a PE stall waiting for the later x chunks
                l1_iter = [(0, 0)]
                l1_iter += [(hi, ci) for hi in range(1, _HI)
                            for ci in range(len(cks))]
                l1_iter += [(0, ci) for ci in range(1, len(cks))]
                w1_tiles = {0: w1_first}
                w1_tiles.update(slot_w1.pop(s))
                for idx, (hi, ci) in enumerate(l1_iter):
                    w1_sb = w1_tiles[hi]
                    t0, tl = cks[ci]
                    ps = ps1.tile([128, tl], F32, tag="ps1")
                    for k in range(_KD):
                        nc.tensor.matmul(
                            ps[:], w1_sb[:, k, :], x_sb[ci][:, k, :],
                            start=(k == 0), stop=(k == _KD - 1),
                        )
                    nc.scalar.activation(
                        h1_sb[:, hi, t0:t0 + tl], ps[:], Relu,
                        bias=b1_sb[:, hi:hi + 1],
                    )
                    if s == 0 and idx % 4 == 3 and idx // 4 < _KD:
                        # slot 0 has no predecessor to prefetch its w2, and
                        # its L2 window is already full streaming slot 1's
                        # weights; spread the w2 pulls through L1's slack
                        stage_w2_di(0, idx // 4)

                # w2 streamed in quarter blocks (8 h-chunks each). All of a
                # slot's w2 is issued in consumption order; the NEXT slot's
                # x, w1, and w2 queue up right behind so even a small
                # (DMA-dense) final slot has its weights buffered ahead of
                # time. The pools throttle the queues to available SBUF.
                if s == 0:
                    for di in range(_KD):
                        if di not in slot_w2.get(0, {}):
                            stage_w2_di(0, di)
                if s + 1 < nslot:
                    emit_slot_prefetch(s + 1)
                    stage_w1(s + 1)
                    stage_w2(s + 1)
                w2_tiles = slot_w2.pop(s)
                for di in range(_KD):
                    w2_sb = w2_tiles.pop(di)
                    for (t0, tl) in cks:
                        ps = ps2.tile([128, tl], F32, tag="ps2")
                        for hi in range(_HI):
                            nc.tensor.matmul(
                                ps[:], w2_sb[hi // 8][:, hi % 8, :],
                                h1_sb[:, hi, t0:t0 + tl],
                                start=(hi == 0), stop=(hi == _HI - 1),
                            )
                        ob = op.tile([128, tl], F32, tag="ob")
                        nc.scalar.activation(
                            ob[:], ps[:], Ident, bias=b2_sb[:, di:di + 1],
                        )
                        nc.sync.dma_start(
                            outT[di, :, off + t0:off + t0 + tl], ob[:]
                        )

    nc.compile()
    _PROG_CACHE[key] = nc
    return nc


def _run(x, gate_w, gate_b, w1, b1, w2, b2, trace=False, trace_cores=None):
    from concourse import bass_utils

    e_sel, scale = _route(x, gate_w, gate_b)
    counts = np.bincount(e_sel, minlength=_E)

    slot_sizes, parts = _pack(counts)
    CAP = sum(slot_sizes)

    # token ids per expert in sorted order
    order = np.argsort(e_sel, kind="stable")
    starts = np.zeros(_E + 1, np.int64)
    np.cumsum(counts, out=starts[1:])
    consumed = [0] * _E

    # slot offsets within a core's token axis
    offs = np.zeros(len(slot_sizes) + 1, np.int64)
    np.cumsum(slot_sizes, out=offs[1:])

    xs = x.reshape(_N, _D) * scale[:, None]  # f32, matches reference scaling

    # prearranged weights, one contiguous block per (expert, chunk):
    # W1L[e, hi, p, k, f] = w1[e, hi*128+f, k*128+p]
    W1L = np.ascontiguousarray(
        w1.reshape(_E, _HI, 128, _KD, 128).transpose(0, 1, 4, 3, 2)
    ).astype(_BF16)
    # W2L[e, di, p, hi, f] = w2[e, di*128+f, hi*128+p]
    W2L = np.ascontiguousarray(
        w2.reshape(_E, _KD, 128, _HI, 128).transpose(0, 1, 4, 3, 2)
    ).astype(_BF16)
    B1L = np.ascontiguousarray(b1.reshape(_E, _HI, 128).transpose(0, 2, 1))
    B2L = np.ascontiguousarray(b2.reshape(_E, _KD, 128).transpose(0, 2, 1))

    slot_expert = [[0] * len(slot_sizes) for _ in range(8)]
    tok_of = np.full((8, CAP), -1, np.int64)
    for (e, cnt, core, si) in parts:
        lo = starts[e] + consumed[e]
        consumed[e] += cnt
        toks = order[lo:lo + cnt]
        tok_of[core, offs[si]:offs[si] + cnt] = toks
        slot_expert[core][si] = e

    in_maps = []
    for core in range(8):
        cols = tok_of[core]
        xsel = np.zeros((CAP, _D), np.float32)
        valid = cols >= 0
        xsel[valid] = xs[cols[valid]]
        XL = np.ascontiguousarray(
            xsel.reshape(CAP, _KD, 128).transpose(2, 1, 0)
        ).astype(_BF16)
        m = {"xt": XL, "wrm": np.zeros((128, 512), _BF16)}
        for si in range(len(slot_sizes)):
            e = slot_expert[core][si]
            m[f"w1_{si}"] = W1L[e]
            m[f"w2_{si}"] = W2L[e]
            m[f"b1_{si}"] = B1L[e]
            m[f"b2_{si}"] = B2L[e]
        in_maps.append(m)

    nc = _build_program(slot_sizes)
    kw = {}
    if trace:
        kw["trace"] = True
        if trace_cores is not None:
            kw["trace_cores"] = trace_cores
    try:
        res = bass_utils.run_bass_kernel_spmd(
            nc, in_maps, core_ids=list(range(8)), **kw
        )
    except Exception:
        # one retry for transient device faults
        import time as _time
        _time.sleep(2.0)
        res = bass_utils.run_bass_kernel_spmd(
            nc, in_maps, core_ids=list(range(8)), **kw
        )

    out = np.zeros((_N, _D), np.float32)
    for core in range(8):
        cols = tok_of[core]
        valid = cols >= 0
        oc = res.results[core]["outT"]  # [KD, 128, CAP]
        ovals = oc.transpose(2, 0, 1).reshape(CAP, _D)
        out[cols[valid]] = ovals[valid]
    return out.reshape(_B, _S, _D), res


def kernel(x, gate_w, gate_b, w1, b1, w2, b2):
    x = np.ascontiguousarray(np.asarray(x, dtype=np.float32))
    gate_w = np.asarray(gate_w, dtype=np.float32)
    gate_b = np.asarray(gate_b, dtype=np.float32)
    w1 = np.ascontiguousarray(np.asarray(w1, dtype=np.float32))
    b1 = np.asarray(b1, dtype=np.float32)
    w2 = np.ascontiguousarray(np.asarray(w2, dtype=np.float32))
    b2 = np.asarray(b2, dtype=np.float32)
    out, _ = _run(x, gate_w, gate_b, w1, b1, w2, b2)
    return out
